# revision 25
# baseline (speedup 1.0000x reference)
"""Trainium2 Bass kernel for nn_LocalExperts (MoE expert-parallel FFN).

Reference computation (per full input):
    x  [T=16384, D=1024] -> reshape [E=8, C=2048, D]
    h  = gelu(x @ w1[e] + b1[e])     w1 [E, D, F=4096]
    y  = h @ w2[e] + b2[e]           w2 [E, F, D]
    out[T, D]

Sharding: expert parallelism across 8 NeuronCores. Expert e's tokens are
exactly rows [e*C:(e+1)*C] of the input, so core e gets that token slice
plus w1[e], b1[e], w2[e], b2[e]. No collectives; outputs are concatenated
on the host.

Host-side marshalling (part of the sharding layout, no FLOPs):
  - X slice is transposed to Xt [D, C] so the contraction dim lands on
    SBUF partitions via plain DMA (kills all PE transposes + staging).
  - X/w1/w2 are cast to bf16 (PE runs 1 cycle/row for bf16, same as
    fp32r, but DMA/SBUF halve and weight loads use FWL). rel-err ~3e-3,
    well inside the 2e-2 gate. b1 is pre-arranged to [128, F/128] so the
    per-f-tile ACT bias column is a contiguous 128B DMA line.

Per-core kernel (C=2048 tokens, one expert), 1536+512-token passes:
  - PE floor is 2048 N=512 bf16 matmuls at ~216.5ns issue-to-issue
    (~443.5us); everything else exists to keep that stream gapless.
  - Xt is resident as per-(512-token-chunk, 4-d-tile) half tiles - Tile
    dependency tracking is tile-granular, so fine tiles let the first
    GEMM1 groups start as soon as their 0.5MB pieces land instead of
    stalling on one big transfer. The first chunk accumulates d 0-3 for
    all four f-tiles (paced with the first two pieces), then d 4-7,
    which arrive exactly one 16-matmul phase later.
  - Per pass, loop F in chunks of FC=512 (w1 on sync / w2 on scalar
    queue, double-buffered; weights re-streamed per pass - DMA idles
    otherwise):
      GEMM1: Ht[f,c] = gelu(W1c.T @ Xt + b1)  (PSUM acc over 8 d-tiles,
                                               ACT gelu drains to bf16)
      GEMM2: Yacc[c,d] += Ht.T @ W2c          (PSUM acc over 4 f-tiles,
                                               DVE acc over 8 chunks)
  - Yacc (fp32, per-pass buffer) initialized with broadcast b2 on chunk
    0. Pass A (1536 tokens) flushes full rows on the sync queue under
    pass B's compute; pass B (512 tokens) flushes half-rows on both
    queues so only ~2MB lands in the post-matmul tail.
  - 6 fp32 identity matmul pairs (~4.2us) bridge the engine preamble to
    the first piece arrival and warm the PE HAM clock to 2.4GHz.
"""

import os
from contextlib import ExitStack

import numpy as np
import ml_dtypes

import concourse.tile as tile
from concourse import bacc
from concourse import mybir
from concourse.bass import ds
from concourse.bass_utils import run_bass_kernel_spmd
from concourse.masks import make_identity

AFT = mybir.ActivationFunctionType

E = 8
D = 1024
F = 4096
T = 16384
C = T // E          # tokens per core
P = 128

CP_A = 1536         # pass A tokens
CP_B = C - CP_A     # pass B tokens (final flush is CP_B*D*4 bytes)
FC = 512            # F chunk per iteration
NFREE = 512         # matmul moving free dim (one PSUM bank of fp32)
D_T = D // P        # 8 d-tiles
DH = D_T // 2       # d-tiles per half piece
FC_T = FC // P      # 4 f-tiles per chunk
N_FC = F // FC      # 8 chunks
N_CC = C // NFREE   # 4 token chunks
N_WARM = 10

MM_MODE = "bf16"    # informational (test.py prints it)
# test-only: CoreSim lacks Gelu; "tanh" swaps the activation for sim gating
ACT_FN = os.environ.get("KERNEL_ACT", "gelu")


def _emit(ctx: ExitStack, tc: tile.TileContext, xt, w1, b1t, w2, b2, y):
    nc = tc.nc
    f32 = mybir.dt.float32
    bf16 = mybir.dt.bfloat16

    consts = ctx.enter_context(tc.tile_pool(name="consts", bufs=1))
    xt_pool = ctx.enter_context(tc.tile_pool(name="xt", bufs=2 * N_CC))
    ya_pool = ctx.enter_context(tc.tile_pool(name="yaccA", bufs=1))
    yb_pool = ctx.enter_context(tc.tile_pool(name="yaccB", bufs=1))
    w1_pool = ctx.enter_context(tc.tile_pool(name="w1c", bufs=2))
    w1h_pool = ctx.enter_context(tc.tile_pool(name="w1h", bufs=2))
    w2_pool = ctx.enter_context(tc.tile_pool(name="w2c", bufs=2))
    ht_pool = ctx.enter_context(tc.tile_pool(name="ht", bufs=2))
    mm_psum = ctx.enter_context(tc.tile_pool(name="mmp", bufs=8, space="PSUM"))

    w1_r = w1.rearrange("(do p) f -> p do f", p=P)    # [128, 8, 4096]
    w2_r = w2.rearrange("(fo p) d -> p fo d", p=P)    # [128, 32, 1024]
    xt_r = xt.rearrange("(dt p) c -> p dt c", p=P)    # [128, 8, 2048]

    # Xt half tiles: xt_sb[cc][h] covers tokens [cc*512, +512), d-tiles
    # [h*4, +4)
    xt_sb = [
        [
            xt_pool.tile([P, DH, NFREE], bf16, tag="xt", name=f"xt{cc}{h}")
            for h in range(2)
        ]
        for cc in range(N_CC)
    ]
    # chunk-0 w1 as two half tiles (piece-granular deps for the start)
    w1c0h = [
        w1h_pool.tile([P, DH, FC], bf16, tag="w1h", name=f"w1c0h{h}")
        for h in range(2)
    ]

    # Startup DMAs in strict need-order, the opener's two pieces leading
    # the two queues; ~0.5-1MB pieces keep 4+ transfers in flight
    # (per-transfer throughput caps at ~100-140GB/s; concurrency buys
    # aggregate bandwidth). b1t rides 3rd on sync - the first ACT needs it
    # one 16-matmul phase after the opener starts.
    nc.scalar.dma_start(xt_sb[0][0][:], xt_r[:, ds(0, DH), ds(0, NFREE)])
    nc.sync.dma_start(w1c0h[0][:], w1_r[:, ds(0, DH), ds(0, FC)])
    nc.scalar.dma_start(w1c0h[1][:], w1_r[:, ds(DH, DH), ds(0, FC)])
    nc.sync.dma_start(xt_sb[0][1][:], xt_r[:, ds(DH, DH), ds(0, NFREE)])
    F_T = F // P
    b1t_sb = consts.tile([P, F_T], f32)
    nc.sync.dma_start(b1t_sb[:], b1t)
    nc.scalar.dma_start(xt_sb[1][0][:], xt_r[:, ds(0, DH), ds(NFREE, NFREE)])
    nc.sync.dma_start(xt_sb[1][1][:], xt_r[:, ds(DH, DH), ds(NFREE, NFREE)])
    nc.scalar.dma_start(
        xt_sb[2][0][:], xt_r[:, ds(0, DH), ds(2 * NFREE, NFREE)]
    )
    nc.sync.dma_start(xt_sb[2][1][:], xt_r[:, ds(DH, DH), ds(2 * NFREE, NFREE)])
    w2c0 = w2_pool.tile([P, FC_T, D], bf16, tag="w2c")
    nc.scalar.dma_start(w2c0[:], w2_r[:, ds(0, FC_T), :])
    nc.sync.dma_start(xt_sb[3][0][:], xt_r[:, ds(0, DH), ds(3 * NFREE, NFREE)])
    nc.scalar.dma_start(
        xt_sb[3][1][:], xt_r[:, ds(DH, DH), ds(3 * NFREE, NFREE)]
    )
    # b2 broadcast across partitions for the Yacc init; needed ~40us in
    b2b = consts.tile([P, D], f32)
    nc.scalar.dma_start(b2b[:], b2[None, :].to_broadcast((P, D)))

    # PE warmup bridging the preamble->first-piece window (each fp32
    # identity matmul is a LOW/HIGH pair, ~840ns cold / ~430ns warm)
    identity = consts.tile([P, P], f32)
    make_identity(nc, identity[:])
    warm_ps = mm_psum.tile([P, NFREE], f32, tag="mm")
    for _ in range(N_WARM):
        nc.tensor.matmul(warm_ps[:, :P], lhsT=identity[:], rhs=identity[:],
                         start=True, stop=True)

    def gemm1(it, pss, fci, c0, CP, w1c):
        # ---- GEMM1: Ht[f, c] = gelu(sum_d W1[d, f]^T Xt[d, c] + b1[f]) ----
        cc0 = c0 // NFREE
        ht = ht_pool.tile([P, FC_T, CP], bf16, tag="ht", name=f"ht{it}")
        for cci in range(CP // NFREE):
            xtc = xt_sb[cc0 + cci]
            if it == 0 and cci == 0:
                # piece-paced opener: d 0-3 for all f-tiles (needs only the
                # first two 0.5MB pieces), then d 4-7 (arriving one 16-MM
                # phase later); the four PSUM groups stay open in between
                pso = [
                    mm_psum.tile([P, NFREE], f32, tag="mm", name=f"pso{i}")
                    for i in range(FC_T)
                ]
                for fti in range(FC_T):
                    for di in range(DH):
                        nc.tensor.matmul(
                            pso[fti][:],
                            lhsT=w1c0h[0][:, di, ds(fti * P, P)],
                            rhs=xtc[0][:, di, :],
                            start=(di == 0),
                            stop=False,
                        )
                for fti in range(FC_T):
                    for di in range(DH):
                        nc.tensor.matmul(
                            pso[fti][:],
                            lhsT=w1c0h[1][:, di, ds(fti * P, P)],
                            rhs=xtc[1][:, di, :],
                            start=False,
                            stop=(di == DH - 1),
                        )
                    nc.scalar.activation(
                        ht[:, fti, ds(0, NFREE)],
                        pso[fti][:],
                        AFT.Tanh if ACT_FN == "tanh" else AFT.Gelu_apprx_tanh,
                        bias=b1t_sb[:, fti : fti + 1],
                        scale=1.0,
                    )
                continue
            for fti in range(FC_T):
                ft_g = fci * FC_T + fti
                ps = mm_psum.tile([P, NFREE], f32, tag="mm")
                for di in range(D_T):
                    if it == 0:
                        lhsT = w1c0h[di // DH][:, di % DH, ds(fti * P, P)]
                    else:
                        lhsT = w1c[:, di, ds(fti * P, P)]
                    nc.tensor.matmul(
                        ps[:],
                        lhsT=lhsT,
                        rhs=xtc[di // DH][:, di % DH, :],
                        start=(di == 0),
                        stop=(di == D_T - 1),
                    )
                nc.scalar.activation(
                    ht[:, fti, ds(cci * NFREE, NFREE)],
                    ps[:],
                    AFT.Tanh if ACT_FN == "tanh" else AFT.Gelu_apprx_tanh,
                    bias=b1t_sb[:, ft_g : ft_g + 1],
                    scale=1.0,
                )
        return ht

    def g2_group(ht, w2c, ci, dci):
        ps = mm_psum.tile([P, NFREE], f32, tag="mm")
        for fti in range(FC_T):
            nc.tensor.matmul(
                ps[:],
                lhsT=ht[:, fti, ds(ci * P, P)],
                rhs=w2c[:, fti, ds(dci * NFREE, NFREE)],
                start=(fti == 0),
                stop=(fti == FC_T - 1),
            )
        return ps

    def final_add_flush(yacc, ps, c0, ci, dci):
        # final value: add + [128,512] DMA on alternating queues. DMA
        # transfers cost ~1.8us fixed latency regardless of size, so one
        # 256KB piece per output tile (4.1us per-queue pacing in the fused
        # window) keeps the queues drained; smaller pieces would saturate
        # the queues on fixed cost and push completions past the last MM.
        ya = yacc[:, ci, ds(dci * NFREE, NFREE)]
        nc.vector.tensor_add(out=ya, in0=ya, in1=ps[:])
        q = nc.scalar if (ci * 2 + dci) % 2 == 0 else nc.sync
        q.dma_start(y[ds(c0 + ci * P, P), ds(dci * NFREE, NFREE)], ya)

    # main loop; the last two f-chunks of pass B are fused (both GEMM1s,
    # then interleaved GEMM2 per output tile) so the final flush spreads
    # over a ~14us window instead of the last chunk's ~7us
    passes = [(0, CP_A), (CP_A, CP_B)]
    w1c, w2c = w1c0h, w2c0
    n_it = 2 * N_FC
    yacc = None
    for it in range(n_it - 1):
        pss, fci = divmod(it, N_FC)
        c0, CP = passes[pss]
        fused = it == n_it - 2

        # prefetch next iteration's weight chunk (double-buffered;
        # w1 on sync, w2 on scalar)
        if it + 1 < n_it:
            nfci = (fci + 1) % N_FC
            w1n = w1_pool.tile([P, D_T, FC], bf16, tag="w1c")
            nc.sync.dma_start(w1n[:], w1_r[:, :, ds(nfci * FC, FC)])
            w2n = w2_pool.tile([P, FC_T, D], bf16, tag="w2c")
            nc.scalar.dma_start(w2n[:], w2_r[:, ds(nfci * FC_T, FC_T), :])

        ht = gemm1(it, pss, fci, c0, CP, w1c)
        if fused:
            ht2 = gemm1(it + 1, pss, fci + 1, c0, CP, w1n)

        # ---- GEMM2: Yacc[c, d] += sum_f Ht[f, c]^T W2[f, d] ----
        if fci == 0:
            pool = ya_pool if pss == 0 else yb_pool
            yacc = pool.tile([P, CP // P, D], f32, tag="yacc")
        for ci in range(CP // P):
            for dci in range(D // NFREE):
                ps = g2_group(ht, w2c, ci, dci)
                ya = yacc[:, ci, ds(dci * NFREE, NFREE)]
                if fci == 0:
                    nc.vector.tensor_add(
                        out=ya, in0=ps[:], in1=b2b[:, ds(dci * NFREE, NFREE)]
                    )
                else:
                    nc.vector.tensor_add(out=ya, in0=ya, in1=ps[:])
                if fused:
                    ps2 = g2_group(ht2, w2n, ci, dci)
                    final_add_flush(yacc, ps2, c0, ci, dci)
            if fci == N_FC - 1 and pss == 0:
                # pass A full-row writebacks, sync queue only: they flush
                # under pass B's compute without ever delaying scalar ACTs
                nc.sync.dma_start(y[ds(c0 + ci * P, P), :], yacc[:, ci, :])
        if it + 1 < n_it:
            w1c, w2c = w1n, w2n


_NC_CACHE = None


def build_bass():
    global _NC_CACHE
    if _NC_CACHE is not None:
        return _NC_CACHE
    nc = bacc.Bacc("TRN2", target_bir_lowering=False, debug=False)
    f32 = mybir.dt.float32
    bf16 = mybir.dt.bfloat16
    xt = nc.dram_tensor("xt", [D, C], bf16, kind="ExternalInput").ap()
    w1 = nc.dram_tensor("w1", [D, F], bf16, kind="ExternalInput").ap()
    b1t = nc.dram_tensor("b1t", [P, F // P], f32, kind="ExternalInput").ap()
    w2 = nc.dram_tensor("w2", [F, D], bf16, kind="ExternalInput").ap()
    b2 = nc.dram_tensor("b2", [D], f32, kind="ExternalInput").ap()
    y = nc.dram_tensor("y", [C, D], f32, kind="ExternalOutput").ap()
    with tile.TileContext(nc) as tc:
        with ExitStack() as ctx:
            _emit(ctx, tc, xt, w1, b1t, w2, b2, y)
    nc.compile()
    _NC_CACHE = nc
    return nc


def _in_maps(inputs, w1, b1, w2, b2):
    bf = ml_dtypes.bfloat16
    maps = []
    for e in range(E):
        xs = inputs[e * C : (e + 1) * C]
        maps.append(
            {
                "xt": np.ascontiguousarray(xs.T).astype(bf),
                "w1": w1[e].astype(bf),
                "b1t": np.ascontiguousarray(
                    b1[e].astype(np.float32).reshape(F // P, P).T
                ),
                "w2": w2[e].astype(bf),
                "b2": np.ascontiguousarray(b2[e], dtype=np.float32),
            }
        )
    return maps


def kernel_run(inputs, w1, b1, w2, b2, trace=False, **trace_kwargs):
    """Run on 8 NeuronCores; returns (full_output [T, D], BassKernelResults)."""
    inputs = np.asarray(inputs, dtype=np.float32)
    w1 = np.asarray(w1, dtype=np.float32)
    b1 = np.asarray(b1, dtype=np.float32)
    w2 = np.asarray(w2, dtype=np.float32)
    b2 = np.asarray(b2, dtype=np.float32)
    nc = build_bass()
    res = run_bass_kernel_spmd(
        nc,
        _in_maps(inputs, w1, b1, w2, b2),
        core_ids=list(range(E)),
        trace=trace,
        **trace_kwargs,
    )
    out = np.concatenate([res.results[e]["y"] for e in range(E)], axis=0)
    return out, res


def kernel(inputs, w1, b1, w2, b2):
    out, _ = kernel_run(inputs, w1, b1, w2, b2, trace=False)
    return out
